# revision 11
# baseline (speedup 1.0000x reference)
"""MoE layer (top-2 of 8 experts, H=1024, FFN=4096) on 8 TRN2 NeuronCores.

Expert-parallel: core e holds expert e's weights. The (tiny) router runs on
host; tokens are gathered per-expert into capacity-padded batches, each core
runs the expert FFN and the host applies gate/b2 and scatter-adds the two
expert contributions per token.

Device layout per core (C = per-expert token capacity, multiple of 4):
  GEMM1  h[f, c] = w1t[h, f].T @ x[h, c]    (F on psum partitions, tokens free)
  GEMM2  y[n, c] = w2t[f, n].T @ h[f, c]    (H on psum partitions, tokens free)
Both GEMMs stream tokens as the moving dimension, so PE cycles scale with the
exact capacity C (no 128-padding of token tiles). y accumulates over the 8
F-slabs in SBUF via DVE adds; out is y in [H, C] orientation and the host
applies out[rows] += gate * (y.T + b2) — gate/b2/combine cost nothing on
device.

GEMMs run in bf16 (fp32 matmul on this PE is 4x slower; fp8 DoubleRow was
measured to stream at the same 1 column/cycle as bf16 — its 2x is per-matmul
contraction, and the residual-corrected variant needed 1.5x the matmuls, so
bf16 is the sweet spot at this error budget). PSUM accumulation, gelu+bias
eviction and the DVE slab accumulation stay fp32. End-to-end absmax-relative
error vs the fp32 reference is ~3.5e-3.

Inputs are host-packed so every DMA is contiguous per partition:
  x blocks  xb{b}: [128, 8, cw_b]      (x.T reshaped, 128-partition rows)
  w1 slabs  packed [8, 4, 128, 8, 128] -> per-(slab,m) piece [128, 8, 128]
  w2 slabs  packed [8, 128, 4, 1024]   -> per-slab tile [128, 4, 1024]
"""

import os

os.environ.setdefault("NEURON_RT_RESET_CORES", "1")

import ml_dtypes
import numpy as np

import concourse.bass as bass  # noqa: F401  (bass types via bacc)
import concourse.mybir as mybir
from concourse import bacc
from concourse.tile import TileContext
from concourse.bass_utils import run_bass_kernel_spmd

H = 1024
E = 8
F = 4096
TOPK = 2
P = 128
N_CORES = 8
NTH = 8            # F slabs
FT = F // NTH      # 512
MF = FT // P       # 4 m-tiles per slab
KH = H // P        # 8 contraction tiles for GEMM1
NHT = H // P       # 8 output H-tiles for GEMM2
FP32 = mybir.dt.float32
BF16 = mybir.dt.bfloat16

_cache: dict = {}

# Test-harness knobs: set TRACE=True before calling kernel() to profile the
# device run; exec time lands in LAST_EXEC_TIME_NS.
TRACE = False
LAST_EXEC_TIME_NS = None


def _blocks(C: int):
    """C-block widths <=512, multiples of 4 (C must be mult of 4).

    The first block is kept small (128) so the first GEMM1 group only waits
    on a small x DMA at kernel start; the rest are near-even.
    """
    widths = []
    rem = C
    if C > 512:
        widths.append(128)
        rem -= 128
    nb = -(-rem // 192)
    q = rem // 4
    units = [q // nb + (1 if i < q % nb else 0) for i in range(nb)]
    widths += [u * 4 for u in units]
    assert sum(widths) == C and all(0 < w <= 512 for w in widths)
    cbs = []
    off = 0
    for w in widths:
        cbs.append((off, w))
        off += w
    return cbs


def _build(C: int):
    """Build + compile the per-core expert-FFN program for capacity C."""
    assert C % 4 == 0
    cbs = _blocks(C)
    nb = len(cbs)

    nc = bacc.Bacc("TRN2", target_bir_lowering=False, debug=False,
                   num_devices=N_CORES)

    xb_d = [nc.dram_tensor(f"xb{b}", [P, KH, cw], BF16, kind="ExternalInput")
            for b, (_, cw) in enumerate(cbs)]
    w1_d = nc.dram_tensor("w1p", [NTH, MF, P, KH, P], BF16, kind="ExternalInput")
    w2_d = nc.dram_tensor("w2p", [NTH, P, MF, H], BF16, kind="ExternalInput")
    b1_d = nc.dram_tensor("b1c", [P, F // P], FP32, kind="ExternalInput")
    out = nc.dram_tensor("out", [H, C], BF16, kind="ExternalOutput")

    GELU = mybir.ActivationFunctionType.Gelu
    ADD = mybir.AluOpType.add

    with TileContext(nc) as tc:
        with (
            tc.tile_pool(name="const", bufs=1) as constp,
            tc.tile_pool(name="xp", bufs=1) as xp,
            tc.tile_pool(name="w1p", bufs=3) as w1p,
            tc.tile_pool(name="w2p", bufs=2) as w2p,
            tc.tile_pool(name="hp", bufs=nb + 1) as hp,
            tc.tile_pool(name="yp", bufs=1) as yp,
            tc.tile_pool(name="ps1", bufs=4, space="PSUM") as ps1,
            tc.tile_pool(name="ps2", bufs=4, space="PSUM") as ps2,
        ):
            # DMA emission order = arrival order: b1 (tiny), w1 slab0 piece
            # m0, x block 0 (small), w1 m1-3, x block 1, w2 slab0, rest of x.
            # The first GEMM1 group can start after ~1MB of DMA.
            b1_sb = constp.tile([P, F // P], FP32, tag="b1")
            nc.sync.dma_start(out=b1_sb[:], in_=b1_d[:])

            def load_w1(th):
                t = w1p.tile([P, MF, KH, P], BF16, tag="w1", name=f"w1_{th}")
                for m in range(MF):
                    nc.sync.dma_start(out=t[:, m, :, :], in_=w1_d[th, m])
                return t

            def load_w2(th):
                t = w2p.tile([P, MF, H], BF16, tag="w2", name=f"w2_{th}")
                nc.sync.dma_start(out=t[:], in_=w2_d[th])
                return t

            x_sb = []

            def load_x(b):
                _, cw = cbs[b]
                t = xp.tile([P, KH, cw], BF16, tag=f"x{b}", name=f"x_{b}")
                nc.sync.dma_start(out=t[:], in_=xb_d[b][:])
                x_sb.append(t)

            w1_cur = w1p.tile([P, MF, KH, P], BF16, tag="w1", name="w1_0")
            nc.sync.dma_start(out=w1_cur[:, 0, :, :], in_=w1_d[0, 0])
            load_x(0)
            for m in range(1, MF):
                nc.sync.dma_start(out=w1_cur[:, m, :, :], in_=w1_d[0, m])
            for b in range(1, nb):
                load_x(b)
            w2_cur = load_w2(0)

            y_sb = yp.tile([P, NHT, C], FP32, tag="y")
            y8 = yp.tile([P, NHT, C], BF16, tag="y8")

            def gemm1(th, b, w1_t):
                coff, cw = cbs[b]
                h_t = hp.tile([P, MF, cw], BF16, tag="h")
                for m in range(MF):
                    pt = ps1.tile([P, cw], FP32, tag="p1")
                    for k in range(KH):
                        nc.tensor.matmul(
                            pt[:], w1_t[:, m, k, :], x_sb[b][:, k, :],
                            start=(k == 0), stop=(k == KH - 1),
                        )
                    nc.scalar.activation(
                        h_t[:, m, :], pt[:], GELU,
                        bias=b1_sb[:, th * MF + m:th * MF + m + 1],
                    )
                return h_t

            def gemm2(th, b, w2_t, h_t):
                coff, cw = cbs[b]
                for ht in range(NHT):
                    pt = ps2.tile([P, cw], FP32, tag="p2")
                    for m in range(MF):
                        nc.tensor.matmul(
                            pt[:], w2_t[:, m, ht * P:(ht + 1) * P], h_t[:, m, :],
                            start=(m == 0), stop=(m == MF - 1),
                        )
                    ys = y_sb[:, ht, coff:coff + cw]
                    if th == 0:
                        nc.vector.tensor_scalar_mul(ys, pt[:], 1.0)
                    elif th < NTH - 1:
                        nc.vector.tensor_tensor(ys, ys, pt[:], ADD)
                    else:
                        # final slab: add in bf16 into the out staging tile
                        # and ship this (ht, block) chunk right away
                        yo = y8[:, ht, coff:coff + cw]
                        nc.vector.tensor_tensor(yo, ys, pt[:], ADD)
                        nc.sync.dma_start(
                            out=out[ht * P:(ht + 1) * P, coff:coff + cw],
                            in_=yo)

            for th in range(NTH):
                w1_t, w2_t = w1_cur, w2_cur
                if th + 1 < NTH:
                    w1_cur = load_w1(th + 1)
                    w2_cur = load_w2(th + 1)
                # all G1 blocks first, then all G2 blocks: the gelu eviction
                # of each block runs under later PE groups, and at startup
                # the w2 slab-0 DMA has until the end of the G1 blocks to
                # arrive.
                h_all = [gemm1(th, b, w1_t) for b in range(nb)]
                for b in range(nb):
                    gemm2(th, b, w2_t, h_all[b])

    nc.compile()
    return nc


def _route(x: np.ndarray, router_w: np.ndarray):
    """Host router: top-2 expert ids + softmax gates per token."""
    logits = x @ router_w.T                                   # [T, E]
    top_i = np.argsort(-logits, axis=1, kind="stable")[:, :TOPK]
    top_v = np.take_along_axis(logits, top_i, axis=1)
    mx = top_v.max(axis=1, keepdims=True)
    ex = np.exp(top_v - mx)
    rw = ex / ex.sum(axis=1, keepdims=True)
    return top_i, rw.astype(np.float32)


def kernel(hidden_states, router_w, w1, b1, w2, b2):
    hidden_states = np.ascontiguousarray(np.asarray(hidden_states, np.float32))
    router_w = np.ascontiguousarray(np.asarray(router_w, np.float32))
    w1 = np.asarray(w1, np.float32)
    b1 = np.asarray(b1, np.float32)
    w2 = np.asarray(w2, np.float32)
    b2 = np.asarray(b2, np.float32)

    B, S, _ = hidden_states.shape
    T = B * S
    x = hidden_states.reshape(T, H)

    top_i, rw = _route(x, router_w)

    sel_idx = []
    sel_gate = []
    for e in range(E):
        mask = top_i == e                                     # [T, K]
        rows = np.nonzero(mask.any(axis=1))[0]
        g = rw[rows[:, None], np.argmax(mask[rows], axis=1)[:, None]][:, 0]
        sel_idx.append(rows)
        sel_gate.append(g.astype(np.float32))

    # One job per (expert, token-chunk). Normally each expert fits in one
    # chunk and a single 8-core SPMD round runs everything; with an extreme
    # routing skew an expert's batch is split into <=C_MAX chunks (bounded
    # by SBUF) and extra rounds run.
    C_MAX = 2048
    jobs = []                                   # (expert, rows, gates)
    for e in range(E):
        rows, g = sel_idx[e], sel_gate[e]
        for off in range(0, max(len(rows), 1), C_MAX):
            jobs.append((e, rows[off:off + C_MAX], g[off:off + C_MAX]))

    n_rounds = -(-len(jobs) // N_CORES)
    cmax = max(len(r) for _, r, _ in jobs)
    C = max(P, -(-cmax // 4) * 4)

    if C not in _cache:
        _cache[C] = _build(C)
    nc = _cache[C]
    cbs = _blocks(C)

    w_bf = {}
    def expert_inputs(e):
        if e not in w_bf:
            # w1[e]: [F, H] -> w1t [H, F] -> [th, m, p, k, pf]
            w1t = w1[e].T.reshape(KH, P, NTH, MF, P)
            w1p = np.ascontiguousarray(
                w1t.transpose(2, 3, 1, 0, 4)).astype(ml_dtypes.bfloat16)
            # w2[e]: [H, F] -> w2t [F, H] -> [th, p, m, H]
            w2t = w2[e].T.reshape(NTH, MF, P, H)
            w2p = np.ascontiguousarray(
                w2t.transpose(0, 2, 1, 3)).astype(ml_dtypes.bfloat16)
            w_bf[e] = {
                "w1p": w1p,
                "w2p": w2p,
                "b1c": np.ascontiguousarray(b1[e].reshape(F // P, P).T),
            }
        return w_bf[e]

    global LAST_EXEC_TIME_NS
    LAST_EXEC_TIME_NS = 0
    out = np.zeros((T, H), np.float32)
    for r in range(n_rounds):
        batch = jobs[r * N_CORES:(r + 1) * N_CORES]
        while len(batch) < N_CORES:
            batch.append((0, sel_idx[0][:0], sel_gate[0][:0]))
        in_maps = []
        for e, rows, g in batch:
            n_e = len(rows)
            xT_e = np.zeros((KH, P, C), np.float32)
            xT_e.reshape(H, C)[:, :n_e] = x[rows].T
            xT_e = xT_e.transpose(1, 0, 2)                    # [P, KH, C]
            im = {**expert_inputs(e)}
            for b, (off, cw) in enumerate(cbs):
                im[f"xb{b}"] = np.ascontiguousarray(
                    xT_e[:, :, off:off + cw]).astype(ml_dtypes.bfloat16)
            in_maps.append(im)

        res = run_bass_kernel_spmd(nc, in_maps, list(range(N_CORES)), trace=TRACE)
        if res.exec_time_ns:
            LAST_EXEC_TIME_NS += res.exec_time_ns

        for core, (e, rows, g) in enumerate(batch):
            if len(rows):
                y = res.results[core]["out"].astype(np.float32)[:, :len(rows)].T
                # row indices are unique within one job, so += is safe
                out[rows] += g[:, None] * (y + b2[e][None, :])

    return out.reshape(B, S, H)


# revision 12
# speedup vs baseline: 1.2858x; 1.2858x over previous
"""MoE layer (top-2 of 8 experts, H=1024, FFN=4096) on 8 TRN2 NeuronCores.

Expert-parallel: core e holds expert e's weights. The (tiny) router runs on
host; tokens are gathered per-expert into capacity-padded batches, each core
runs the expert FFN and the host applies gate/b2 and scatter-adds the two
expert contributions per token.

Device layout per core (C = per-expert token capacity, multiple of 4):
  GEMM1  h[f, c] = w1t[h, f].T @ x[h, c]    (F on psum partitions, tokens free)
  GEMM2  y[n, c] = w2t[f, n].T @ h[f, c]    (H on psum partitions, tokens free)
Both GEMMs stream tokens as the moving dimension, so PE cycles scale with the
exact capacity C (no 128-padding of token tiles). y accumulates over the 8
F-slabs in SBUF via DVE adds; out is y in [H, C] orientation and the host
applies out[rows] += gate * (y.T + b2) — gate/b2/combine cost nothing on
device.

GEMMs run in bf16 (fp32 matmul on this PE is 4x slower; fp8 DoubleRow was
measured to stream at the same 1 column/cycle as bf16 — its 2x is per-matmul
contraction, and the residual-corrected variant needed 1.5x the matmuls, so
bf16 is the sweet spot at this error budget). PSUM accumulation, gelu+bias
eviction and the DVE slab accumulation stay fp32. End-to-end absmax-relative
error vs the fp32 reference is ~3.5e-3.

Inputs are host-packed so every DMA is contiguous per partition:
  x blocks  xb{b}: [128, 8, cw_b]      (x.T reshaped, 128-partition rows)
  w1 slabs  packed [8, 4, 128, 8, 128] -> per-(slab,m) piece [128, 8, 128]
  w2 slabs  packed [8, 128, 4, 1024]   -> per-slab tile [128, 4, 1024]
"""

import os

os.environ.setdefault("NEURON_RT_RESET_CORES", "1")

import ml_dtypes
import numpy as np

import concourse.bass as bass  # noqa: F401  (bass types via bacc)
import concourse.mybir as mybir
from concourse import bacc
from concourse.tile import TileContext
from concourse.bass_utils import run_bass_kernel_spmd

H = 1024
E = 8
F = 4096
TOPK = 2
P = 128
N_CORES = 8
NTH = 8            # F slabs
FT = F // NTH      # 512
MF = FT // P       # 4 m-tiles per slab
KH = H // P        # 8 contraction tiles for GEMM1
NHT = H // P       # 8 output H-tiles for GEMM2
FP32 = mybir.dt.float32
BF16 = mybir.dt.bfloat16

_cache: dict = {}

# Test-harness knobs: set TRACE=True before calling kernel() to profile the
# device run; exec time lands in LAST_EXEC_TIME_NS.
TRACE = False
LAST_EXEC_TIME_NS = None


def _blocks(C: int):
    """C-block widths <=512, multiples of 4 (C must be mult of 4).

    The first block is kept small (128) so the first GEMM1 group only waits
    on a small x DMA at kernel start; the rest are near-even.
    """
    widths = []
    rem = C
    if C > 512:
        widths.append(128)
        rem -= 128
    nb = -(-rem // 512)
    q = rem // 4
    units = [q // nb + (1 if i < q % nb else 0) for i in range(nb)]
    widths += [u * 4 for u in units]
    assert sum(widths) == C and all(0 < w <= 512 for w in widths)
    cbs = []
    off = 0
    for w in widths:
        cbs.append((off, w))
        off += w
    return cbs


def _build(C: int):
    """Build + compile the per-core expert-FFN program for capacity C."""
    assert C % 4 == 0
    cbs = _blocks(C)
    nb = len(cbs)

    nc = bacc.Bacc("TRN2", target_bir_lowering=False, debug=False,
                   num_devices=N_CORES)

    xb_d = [nc.dram_tensor(f"xb{b}", [P, KH, cw], BF16, kind="ExternalInput")
            for b, (_, cw) in enumerate(cbs)]
    w1_d = nc.dram_tensor("w1p", [NTH, MF, P, KH, P], BF16, kind="ExternalInput")
    w2_d = nc.dram_tensor("w2p", [NTH, P, MF, H], BF16, kind="ExternalInput")
    b1_d = nc.dram_tensor("b1c", [P, F // P], FP32, kind="ExternalInput")
    out = nc.dram_tensor("out", [H, C], BF16, kind="ExternalOutput")

    GELU = mybir.ActivationFunctionType.Gelu
    ADD = mybir.AluOpType.add

    with TileContext(nc) as tc:
        with (
            tc.tile_pool(name="const", bufs=1) as constp,
            tc.tile_pool(name="xp", bufs=1) as xp,
            tc.tile_pool(name="w1p", bufs=3) as w1p,
            tc.tile_pool(name="w2p", bufs=2) as w2p,
            tc.tile_pool(name="hp", bufs=nb + 1) as hp,
            tc.tile_pool(name="yp", bufs=1) as yp,
            tc.tile_pool(name="ps1", bufs=4, space="PSUM") as ps1,
            tc.tile_pool(name="ps2", bufs=4, space="PSUM") as ps2,
        ):
            # DMA emission order = arrival order: b1 (tiny), w1 slab0 piece
            # m0, x block 0 (small), w1 m1-3, x block 1, w2 slab0, rest of x.
            # The first GEMM1 group can start after ~1MB of DMA.
            b1_sb = constp.tile([P, F // P], FP32, tag="b1")
            nc.sync.dma_start(out=b1_sb[:], in_=b1_d[:])

            def load_w1(th):
                t = w1p.tile([P, MF, KH, P], BF16, tag="w1", name=f"w1_{th}")
                for m in range(MF):
                    nc.sync.dma_start(out=t[:, m, :, :], in_=w1_d[th, m])
                return t

            def load_w2(th):
                t = w2p.tile([P, MF, H], BF16, tag="w2", name=f"w2_{th}")
                nc.sync.dma_start(out=t[:], in_=w2_d[th])
                return t

            x_sb = []

            def load_x(b):
                _, cw = cbs[b]
                t = xp.tile([P, KH, cw], BF16, tag=f"x{b}", name=f"x_{b}")
                nc.sync.dma_start(out=t[:], in_=xb_d[b][:])
                x_sb.append(t)

            w1_cur = w1p.tile([P, MF, KH, P], BF16, tag="w1", name="w1_0")
            nc.sync.dma_start(out=w1_cur[:, 0, :, :], in_=w1_d[0, 0])
            load_x(0)
            for m in range(1, MF):
                nc.sync.dma_start(out=w1_cur[:, m, :, :], in_=w1_d[0, m])
            if nb > 1:
                load_x(1)
            w2_cur = load_w2(0)
            for b in range(2, nb):
                load_x(b)

            y_sb = yp.tile([P, NHT, C], FP32, tag="y")
            y8 = yp.tile([P, NHT, C], BF16, tag="y8")

            def gemm1(th, b, w1_t):
                coff, cw = cbs[b]
                h_t = hp.tile([P, MF, cw], BF16, tag="h")
                for m in range(MF):
                    pt = ps1.tile([P, cw], FP32, tag="p1")
                    for k in range(KH):
                        nc.tensor.matmul(
                            pt[:], w1_t[:, m, k, :], x_sb[b][:, k, :],
                            start=(k == 0), stop=(k == KH - 1),
                        )
                    nc.scalar.activation(
                        h_t[:, m, :], pt[:], GELU,
                        bias=b1_sb[:, th * MF + m:th * MF + m + 1],
                    )
                return h_t

            def gemm2(th, b, w2_t, h_t):
                coff, cw = cbs[b]
                for ht in range(NHT):
                    pt = ps2.tile([P, cw], FP32, tag="p2")
                    for m in range(MF):
                        nc.tensor.matmul(
                            pt[:], w2_t[:, m, ht * P:(ht + 1) * P], h_t[:, m, :],
                            start=(m == 0), stop=(m == MF - 1),
                        )
                    ys = y_sb[:, ht, coff:coff + cw]
                    if th == 0:
                        nc.vector.tensor_scalar_mul(ys, pt[:], 1.0)
                    elif th < NTH - 1:
                        nc.vector.tensor_tensor(ys, ys, pt[:], ADD)
                    else:
                        # final slab: add in bf16 into the out staging tile
                        # and ship this (ht, block) chunk right away
                        yo = y8[:, ht, coff:coff + cw]
                        nc.vector.tensor_tensor(yo, ys, pt[:], ADD)
                        nc.sync.dma_start(
                            out=out[ht * P:(ht + 1) * P, coff:coff + cw],
                            in_=yo)

            for th in range(NTH):
                w1_t, w2_t = w1_cur, w2_cur
                if th + 1 < NTH:
                    w1_cur = load_w1(th + 1)
                    w2_cur = load_w2(th + 1)
                # software-pipelined order: G1(b0) G1(b1) G2(b0) ... so the
                # gelu eviction of block b runs under the PE mms of the next
                # G1/G2 group instead of stalling GEMM2.
                h_prev = gemm1(th, 0, w1_t)
                for b in range(1, nb):
                    h_b = gemm1(th, b, w1_t)
                    gemm2(th, b - 1, w2_t, h_prev)
                    h_prev = h_b
                gemm2(th, nb - 1, w2_t, h_prev)

    nc.compile()
    return nc


def _route(x: np.ndarray, router_w: np.ndarray):
    """Host router: top-2 expert ids + softmax gates per token."""
    logits = x @ router_w.T                                   # [T, E]
    top_i = np.argsort(-logits, axis=1, kind="stable")[:, :TOPK]
    top_v = np.take_along_axis(logits, top_i, axis=1)
    mx = top_v.max(axis=1, keepdims=True)
    ex = np.exp(top_v - mx)
    rw = ex / ex.sum(axis=1, keepdims=True)
    return top_i, rw.astype(np.float32)


def kernel(hidden_states, router_w, w1, b1, w2, b2):
    hidden_states = np.ascontiguousarray(np.asarray(hidden_states, np.float32))
    router_w = np.ascontiguousarray(np.asarray(router_w, np.float32))
    w1 = np.asarray(w1, np.float32)
    b1 = np.asarray(b1, np.float32)
    w2 = np.asarray(w2, np.float32)
    b2 = np.asarray(b2, np.float32)

    B, S, _ = hidden_states.shape
    T = B * S
    x = hidden_states.reshape(T, H)

    top_i, rw = _route(x, router_w)

    sel_idx = []
    sel_gate = []
    for e in range(E):
        mask = top_i == e                                     # [T, K]
        rows = np.nonzero(mask.any(axis=1))[0]
        g = rw[rows[:, None], np.argmax(mask[rows], axis=1)[:, None]][:, 0]
        sel_idx.append(rows)
        sel_gate.append(g.astype(np.float32))

    # One job per (expert, token-chunk). Normally each expert fits in one
    # chunk and a single 8-core SPMD round runs everything; with an extreme
    # routing skew an expert's batch is split into <=C_MAX chunks (bounded
    # by SBUF) and extra rounds run.
    C_MAX = 2048
    jobs = []                                   # (expert, rows, gates)
    for e in range(E):
        rows, g = sel_idx[e], sel_gate[e]
        for off in range(0, max(len(rows), 1), C_MAX):
            jobs.append((e, rows[off:off + C_MAX], g[off:off + C_MAX]))

    n_rounds = -(-len(jobs) // N_CORES)
    cmax = max(len(r) for _, r, _ in jobs)
    C = max(P, -(-cmax // 4) * 4)

    if C not in _cache:
        _cache[C] = _build(C)
    nc = _cache[C]
    cbs = _blocks(C)

    w_bf = {}
    def expert_inputs(e):
        if e not in w_bf:
            # w1[e]: [F, H] -> w1t [H, F] -> [th, m, p, k, pf]
            w1t = w1[e].T.reshape(KH, P, NTH, MF, P)
            w1p = np.ascontiguousarray(
                w1t.transpose(2, 3, 1, 0, 4)).astype(ml_dtypes.bfloat16)
            # w2[e]: [H, F] -> w2t [F, H] -> [th, p, m, H]
            w2t = w2[e].T.reshape(NTH, MF, P, H)
            w2p = np.ascontiguousarray(
                w2t.transpose(0, 2, 1, 3)).astype(ml_dtypes.bfloat16)
            w_bf[e] = {
                "w1p": w1p,
                "w2p": w2p,
                "b1c": np.ascontiguousarray(b1[e].reshape(F // P, P).T),
            }
        return w_bf[e]

    global LAST_EXEC_TIME_NS
    LAST_EXEC_TIME_NS = 0
    out = np.zeros((T, H), np.float32)
    for r in range(n_rounds):
        batch = jobs[r * N_CORES:(r + 1) * N_CORES]
        while len(batch) < N_CORES:
            batch.append((0, sel_idx[0][:0], sel_gate[0][:0]))
        in_maps = []
        for e, rows, g in batch:
            n_e = len(rows)
            xT_e = np.zeros((KH, P, C), np.float32)
            xT_e.reshape(H, C)[:, :n_e] = x[rows].T
            xT_e = xT_e.transpose(1, 0, 2)                    # [P, KH, C]
            im = {**expert_inputs(e)}
            for b, (off, cw) in enumerate(cbs):
                im[f"xb{b}"] = np.ascontiguousarray(
                    xT_e[:, :, off:off + cw]).astype(ml_dtypes.bfloat16)
            in_maps.append(im)

        res = run_bass_kernel_spmd(nc, in_maps, list(range(N_CORES)), trace=TRACE)
        if res.exec_time_ns:
            LAST_EXEC_TIME_NS += res.exec_time_ns

        for core, (e, rows, g) in enumerate(batch):
            if len(rows):
                y = res.results[core]["out"].astype(np.float32)[:, :len(rows)].T
                # row indices are unique within one job, so += is safe
                out[rows] += g[:, None] * (y + b2[e][None, :])

    return out.reshape(B, S, H)


# revision 13
# speedup vs baseline: 1.2903x; 1.0035x over previous
"""MoE layer (top-2 of 8 experts, H=1024, FFN=4096) on 8 TRN2 NeuronCores.

Expert-parallel: core e holds expert e's weights. The (tiny) router runs on
host; tokens are gathered per-expert into capacity-padded batches, each core
runs the expert FFN and the host applies gate/b2 and scatter-adds the two
expert contributions per token.

Device layout per core (C = per-expert token capacity, multiple of 4):
  GEMM1  h[f, c] = w1t[h, f].T @ x[h, c]    (F on psum partitions, tokens free)
  GEMM2  y[n, c] = w2t[f, n].T @ h[f, c]    (H on psum partitions, tokens free)
Both GEMMs stream tokens as the moving dimension, so PE cycles scale with the
exact capacity C (no 128-padding of token tiles). y accumulates over the 8
F-slabs in SBUF via DVE adds; out is y in [H, C] orientation and the host
applies out[rows] += gate * (y.T + b2) — gate/b2/combine cost nothing on
device.

GEMMs run in bf16 (fp32 matmul on this PE is 4x slower; fp8 DoubleRow was
measured to stream at the same 1 column/cycle as bf16 — its 2x is per-matmul
contraction, and the residual-corrected variant needed 1.5x the matmuls, so
bf16 is the sweet spot at this error budget). PSUM accumulation, gelu+bias
eviction and the DVE slab accumulation stay fp32. End-to-end absmax-relative
error vs the fp32 reference is ~3.5e-3.

Inputs are host-packed so every DMA is contiguous per partition:
  x blocks  xb{b}: [128, 8, cw_b]      (x.T reshaped, 128-partition rows)
  w1 slabs  packed [8, 4, 128, 8, 128] -> per-(slab,m) piece [128, 8, 128]
  w2 slabs  packed [8, 128, 4, 1024]   -> per-slab tile [128, 4, 1024]
"""

import os

os.environ.setdefault("NEURON_RT_RESET_CORES", "1")

import ml_dtypes
import numpy as np

import concourse.bass as bass  # noqa: F401  (bass types via bacc)
import concourse.mybir as mybir
from concourse import bacc
from concourse.tile import TileContext
from concourse.bass_utils import run_bass_kernel_spmd

H = 1024
E = 8
F = 4096
TOPK = 2
P = 128
N_CORES = 8
NTH = 8            # F slabs
FT = F // NTH      # 512
MF = FT // P       # 4 m-tiles per slab
KH = H // P        # 8 contraction tiles for GEMM1
NHT = H // P       # 8 output H-tiles for GEMM2
FP32 = mybir.dt.float32
BF16 = mybir.dt.bfloat16

_cache: dict = {}

# Test-harness knobs: set TRACE=True before calling kernel() to profile the
# device run; exec time lands in LAST_EXEC_TIME_NS.
TRACE = False
LAST_EXEC_TIME_NS = None


def _blocks(C: int):
    """C-block widths <=512, multiples of 4 (C must be mult of 4).

    The first block is kept small (128) so the first GEMM1 group only waits
    on a small x DMA at kernel start; the rest are near-even.
    """
    widths = []
    rem = C
    if C > 512:
        widths.append(128)
        rem -= 128
    nb = -(-rem // 512)
    q = rem // 4
    units = [q // nb + (1 if i < q % nb else 0) for i in range(nb)]
    widths += [u * 4 for u in units]
    assert sum(widths) == C and all(0 < w <= 512 for w in widths)
    cbs = []
    off = 0
    for w in widths:
        cbs.append((off, w))
        off += w
    return cbs


def _build(C: int):
    """Build + compile the per-core expert-FFN program for capacity C."""
    assert C % 4 == 0
    cbs = _blocks(C)
    nb = len(cbs)

    nc = bacc.Bacc("TRN2", target_bir_lowering=False, debug=False,
                   num_devices=N_CORES)

    xb_d = [nc.dram_tensor(f"xb{b}", [P, KH, cw], BF16, kind="ExternalInput")
            for b, (_, cw) in enumerate(cbs)]
    w1_d = nc.dram_tensor("w1p", [NTH, MF, P, KH, P], BF16, kind="ExternalInput")
    w2_d = nc.dram_tensor("w2p", [NTH, P, MF, H], BF16, kind="ExternalInput")
    b1_d = nc.dram_tensor("b1c", [P, F // P], FP32, kind="ExternalInput")
    out = nc.dram_tensor("out", [H, C], BF16, kind="ExternalOutput")

    GELU = mybir.ActivationFunctionType.Gelu
    ADD = mybir.AluOpType.add

    with TileContext(nc) as tc:
        with (
            tc.tile_pool(name="const", bufs=1) as constp,
            tc.tile_pool(name="xp", bufs=1) as xp,
            tc.tile_pool(name="w1p", bufs=3) as w1p,
            tc.tile_pool(name="w2p", bufs=2) as w2p,
            tc.tile_pool(name="hp", bufs=nb + 1) as hp,
            tc.tile_pool(name="yp", bufs=1) as yp,
            tc.tile_pool(name="ps1", bufs=3, space="PSUM") as ps1,
            tc.tile_pool(name="ps2", bufs=5, space="PSUM") as ps2,
        ):
            # DMA emission order = arrival order: b1 (tiny), w1 slab0 piece
            # m0, x block 0 (small), w1 m1-3, x block 1, w2 slab0, rest of x.
            # The first GEMM1 group can start after ~1MB of DMA.
            b1_sb = constp.tile([P, F // P], FP32, tag="b1")
            nc.sync.dma_start(out=b1_sb[:], in_=b1_d[:])

            def load_w1(th):
                t = w1p.tile([P, MF, KH, P], BF16, tag="w1", name=f"w1_{th}")
                for m in range(MF):
                    nc.sync.dma_start(out=t[:, m, :, :], in_=w1_d[th, m])
                return t

            def load_w2(th):
                t = w2p.tile([P, MF, H], BF16, tag="w2", name=f"w2_{th}")
                nc.sync.dma_start(out=t[:], in_=w2_d[th])
                return t

            x_sb = []

            def load_x(b):
                _, cw = cbs[b]
                t = xp.tile([P, KH, cw], BF16, tag=f"x{b}", name=f"x_{b}")
                nc.sync.dma_start(out=t[:], in_=xb_d[b][:])
                x_sb.append(t)

            w1_cur = w1p.tile([P, MF, KH, P], BF16, tag="w1", name="w1_0")
            nc.sync.dma_start(out=w1_cur[:, 0, :, :], in_=w1_d[0, 0])
            load_x(0)
            for m in range(1, MF):
                nc.sync.dma_start(out=w1_cur[:, m, :, :], in_=w1_d[0, m])
            for b in range(1, nb):
                load_x(b)
            w2_cur = load_w2(0)

            y_sb = yp.tile([P, NHT, C], FP32, tag="y")
            y8 = yp.tile([P, NHT, C], BF16, tag="y8")

            def gemm1(th, b, w1_t):
                coff, cw = cbs[b]
                h_t = hp.tile([P, MF, cw], BF16, tag="h")
                for m in range(MF):
                    pt = ps1.tile([P, cw], FP32, tag="p1")
                    for k in range(KH):
                        nc.tensor.matmul(
                            pt[:], w1_t[:, m, k, :], x_sb[b][:, k, :],
                            start=(k == 0), stop=(k == KH - 1),
                        )
                    nc.scalar.activation(
                        h_t[:, m, :], pt[:], GELU,
                        bias=b1_sb[:, th * MF + m:th * MF + m + 1],
                    )
                return h_t

            def gemm2(th, b, w2_t, h_t):
                coff, cw = cbs[b]
                for ht in range(NHT):
                    pt = ps2.tile([P, cw], FP32, tag="p2")
                    for m in range(MF):
                        nc.tensor.matmul(
                            pt[:], w2_t[:, m, ht * P:(ht + 1) * P], h_t[:, m, :],
                            start=(m == 0), stop=(m == MF - 1),
                        )
                    ys = y_sb[:, ht, coff:coff + cw]
                    if th == 0:
                        nc.vector.tensor_scalar_mul(ys, pt[:], 1.0)
                    elif th < NTH - 1:
                        nc.vector.tensor_tensor(ys, ys, pt[:], ADD)
                    else:
                        # final slab: add in bf16 into the out staging tile
                        # and ship this (ht, block) chunk right away
                        yo = y8[:, ht, coff:coff + cw]
                        nc.vector.tensor_tensor(yo, ys, pt[:], ADD)
                        nc.sync.dma_start(
                            out=out[ht * P:(ht + 1) * P, coff:coff + cw],
                            in_=yo)

            for th in range(NTH):
                w1_t, w2_t = w1_cur, w2_cur
                if th + 1 < NTH:
                    w1_cur = load_w1(th + 1)
                    w2_cur = load_w2(th + 1)
                # software-pipelined order: G1(b0) G1(b1) G2(b0) ... so the
                # gelu eviction of block b runs under the PE mms of the next
                # G1/G2 group instead of stalling GEMM2.
                h_prev = gemm1(th, 0, w1_t)
                for b in range(1, nb):
                    h_b = gemm1(th, b, w1_t)
                    gemm2(th, b - 1, w2_t, h_prev)
                    h_prev = h_b
                gemm2(th, nb - 1, w2_t, h_prev)

    nc.compile()
    return nc


def _route(x: np.ndarray, router_w: np.ndarray):
    """Host router: top-2 expert ids + softmax gates per token."""
    logits = x @ router_w.T                                   # [T, E]
    top_i = np.argsort(-logits, axis=1, kind="stable")[:, :TOPK]
    top_v = np.take_along_axis(logits, top_i, axis=1)
    mx = top_v.max(axis=1, keepdims=True)
    ex = np.exp(top_v - mx)
    rw = ex / ex.sum(axis=1, keepdims=True)
    return top_i, rw.astype(np.float32)


def kernel(hidden_states, router_w, w1, b1, w2, b2):
    hidden_states = np.ascontiguousarray(np.asarray(hidden_states, np.float32))
    router_w = np.ascontiguousarray(np.asarray(router_w, np.float32))
    w1 = np.asarray(w1, np.float32)
    b1 = np.asarray(b1, np.float32)
    w2 = np.asarray(w2, np.float32)
    b2 = np.asarray(b2, np.float32)

    B, S, _ = hidden_states.shape
    T = B * S
    x = hidden_states.reshape(T, H)

    top_i, rw = _route(x, router_w)

    sel_idx = []
    sel_gate = []
    for e in range(E):
        mask = top_i == e                                     # [T, K]
        rows = np.nonzero(mask.any(axis=1))[0]
        g = rw[rows[:, None], np.argmax(mask[rows], axis=1)[:, None]][:, 0]
        sel_idx.append(rows)
        sel_gate.append(g.astype(np.float32))

    # One job per (expert, token-chunk). Normally each expert fits in one
    # chunk and a single 8-core SPMD round runs everything; with an extreme
    # routing skew an expert's batch is split into <=C_MAX chunks (bounded
    # by SBUF) and extra rounds run.
    C_MAX = 2048
    jobs = []                                   # (expert, rows, gates)
    for e in range(E):
        rows, g = sel_idx[e], sel_gate[e]
        for off in range(0, max(len(rows), 1), C_MAX):
            jobs.append((e, rows[off:off + C_MAX], g[off:off + C_MAX]))

    n_rounds = -(-len(jobs) // N_CORES)
    cmax = max(len(r) for _, r, _ in jobs)
    C = max(P, -(-cmax // 4) * 4)

    if C not in _cache:
        _cache[C] = _build(C)
    nc = _cache[C]
    cbs = _blocks(C)

    w_bf = {}
    def expert_inputs(e):
        if e not in w_bf:
            # w1[e]: [F, H] -> w1t [H, F] -> [th, m, p, k, pf]
            w1t = w1[e].T.reshape(KH, P, NTH, MF, P)
            w1p = np.ascontiguousarray(
                w1t.transpose(2, 3, 1, 0, 4)).astype(ml_dtypes.bfloat16)
            # w2[e]: [H, F] -> w2t [F, H] -> [th, p, m, H]
            w2t = w2[e].T.reshape(NTH, MF, P, H)
            w2p = np.ascontiguousarray(
                w2t.transpose(0, 2, 1, 3)).astype(ml_dtypes.bfloat16)
            w_bf[e] = {
                "w1p": w1p,
                "w2p": w2p,
                "b1c": np.ascontiguousarray(b1[e].reshape(F // P, P).T),
            }
        return w_bf[e]

    global LAST_EXEC_TIME_NS
    LAST_EXEC_TIME_NS = 0
    out = np.zeros((T, H), np.float32)
    for r in range(n_rounds):
        batch = jobs[r * N_CORES:(r + 1) * N_CORES]
        while len(batch) < N_CORES:
            batch.append((0, sel_idx[0][:0], sel_gate[0][:0]))
        in_maps = []
        for e, rows, g in batch:
            n_e = len(rows)
            xT_e = np.zeros((KH, P, C), np.float32)
            xT_e.reshape(H, C)[:, :n_e] = x[rows].T
            xT_e = xT_e.transpose(1, 0, 2)                    # [P, KH, C]
            im = {**expert_inputs(e)}
            for b, (off, cw) in enumerate(cbs):
                im[f"xb{b}"] = np.ascontiguousarray(
                    xT_e[:, :, off:off + cw]).astype(ml_dtypes.bfloat16)
            in_maps.append(im)

        res = run_bass_kernel_spmd(nc, in_maps, list(range(N_CORES)), trace=TRACE)
        if res.exec_time_ns:
            LAST_EXEC_TIME_NS += res.exec_time_ns

        for core, (e, rows, g) in enumerate(batch):
            if len(rows):
                y = res.results[core]["out"].astype(np.float32)[:, :len(rows)].T
                # row indices are unique within one job, so += is safe
                out[rows] += g[:, None] * (y + b2[e][None, :])

    return out.reshape(B, S, H)
